# revision 43
# baseline (speedup 1.0000x reference)
"""Trainium2 Bass kernel for a 2-layer dense GCN block:

    z = x.reshape(B, N, F)                     # B=4, N=8192, F=64
    for i in range(2):
        z = relu((A @ z) @ W_i)                # A: [N, N] dense
    return z

Strategy (8 NeuronCores, SPMD):
  * Shard the output rows (m) of A @ Z across cores: core j owns rows
    [1024*j, 1024*(j+1)) and keeps its A^T column-slice (bf16, 16 MiB)
    resident in SBUF for BOTH layers, so A is read from HBM exactly once.
  * HOST-SIDE REPACK: A^T / Z0 / the output all use per-partition
    contiguous tiled layouts so every DMA line is a 1-8 KiB run (the
    naive [n, m] layout caps the shared hardware DGE at ~90 GB/s on
    2 KiB descriptor lines — measured).  A^T is split into two m-half
    streams, mh=0 first, all on one queue in exact consumption order.
  * Z is a [n, c] matrix with c = b*F + f (256 columns).  Layer matmuls
    compute H^T[c, m] = sum_n Z[n, c] * A^T[n, m] (lhsT = Z tile,
    rhs = A^T tile, fp32 PSUM accum).
  * Layer 1 runs as TWO m-half passes in DMA arrival order.  Each
    half's tail (per-m-tile PSUM copy, weight apply via a block-diagonal
    diag(W,W) tile, relu into a contiguous send tile, one 2 KiB-line
    store, one 2 MiB AllGather) is pinned ahead of the next pass.  The
    mh=0 store DMA is queued BETWEEN the two A^T load streams so its
    packets do not trail the 20 MiB load on the shared DGE engine.
  * The two gathers pipeline on the single CC stream right after the
    runtime's kernel barrier clears (~75us; the first gather lands
    ~95-115us — this is the hard critical path, so layer 2 starts then).
  * Gathered Z1 is staged per core-block into resident SBUF tiles with
    2 KiB-line DMAs; layer 2's n-loop follows gather arrival order and
    its final 16 n-tiles run as two m-half sweeps so the output tail
    overlaps accumulation.
  * bf16 operands / fp32 accumulation (measured ~0.5% rel-l2 vs the
    fp32 reference).  Final output is fp32 (reassembled on the host).
"""

import contextlib

import numpy as np
import ml_dtypes

import concourse.mybir as mybir
import concourse.tile as tile
from concourse import bacc
from concourse.bass_utils import run_bass_kernel_spmd

BF16 = ml_dtypes.bfloat16

NCORES = 8
B, N, F, L = 4, 8192, 64, 2
C = B * F                      # 256 columns of the Z matrix
M_CORE = N // NCORES           # 1024 output rows per core
NT = N // 128                  # 64 contraction tiles of 128
MT = M_CORE // 128             # 8 output-row tiles of 128 per core
NG = 2                         # one AllGather per m-half
MPG = MT // NG                 # m-tiles per gather slice (4)
TPC = 8                        # n-tiles per DMA chunk
KCH = NT // TPC                # 8 chunks
STAG2 = 16                     # layer-2 tail sweep tiles

_CACHED = {}


def _build_program():
    nc = bacc.Bacc("TRN2", target_bir_lowering=False, debug=False,
                   num_devices=NCORES)
    dt = mybir.dt

    # host-repacked inputs: per-partition-contiguous tiled layouts
    atr_d = nc.dram_tensor("atr", [2 * KCH, 128, TPC * 512], dt.bfloat16,
                           kind="ExternalInput")
    z0r_d = nc.dram_tensor("z0r", [KCH, 128, TPC * C], dt.bfloat16,
                           kind="ExternalInput")
    w_d = nc.dram_tensor("w", [128, 2 * 128], dt.bfloat16, kind="ExternalInput")
    # output in [partition, tile, c] layout; host reassembles
    out_d = nc.dram_tensor("out", [128, MT * C], dt.bfloat16,
                           kind="ExternalOutput")

    z1_loc = [nc.dram_tensor(f"z1_loc{g}", [128, MPG * C], dt.bfloat16)
              for g in range(NG)]
    z1g = [nc.dram_tensor(f"z1g{g}", [NCORES * 128, MPG * C], dt.bfloat16,
                          addr_space="Shared")
           for g in range(NG)]

    with tile.TileContext(nc) as tc:
        with tc.tile_pool(name="a_res", bufs=1) as a_pool, \
             tc.tile_pool(name="z_res", bufs=1) as z_pool, \
             tc.tile_pool(name="z1_res", bufs=1) as z1_pool, \
             tc.tile_pool(name="snd", bufs=1) as snd_pool, \
             tc.tile_pool(name="wk", bufs=1) as w_pool, \
             tc.tile_pool(name="ht", bufs=3, space="PSUM") as psh_pool, \
             tc.tile_pool(name="pz", bufs=2, space="PSUM") as psz_pool, \
             tc.tile_pool(name="hsb", bufs=2) as hsb_pool:

            w_sb = w_pool.tile([128, 2 * 128], dt.bfloat16, tag="w")
            nc.scalar.dma_start(out=w_sb[:], in_=w_d[:])

            ath_sb = {(mh, j): a_pool.tile([128, TPC * 512], dt.bfloat16,
                                           tag=f"ath{mh}{j}",
                                           name=f"ath_sb{mh}{j}")
                      for mh in range(2) for j in range(KCH)}
            z_sb = [z_pool.tile([128, TPC * C], dt.bfloat16,
                                tag=f"z{j}", name=f"z_sb{j}")
                    for j in range(KCH)]
            z1_sb = [z1_pool.tile([128, NCORES * MPG * C], dt.bfloat16,
                                  tag=f"z1s{g}", name=f"z1_sb{g}")
                     for g in range(NG)]
            # contiguous per-half send tiles (relu writes slices of these)
            z1snd = [snd_pool.tile([128, MPG * C], dt.bfloat16,
                                   tag=f"z1snd{mh}", name=f"z1snd{mh}")
                     for mh in range(2)]
            z2snd = [snd_pool.tile([128, MPG * C], dt.bfloat16,
                                   tag=f"z2snd{mh}", name=f"z2snd{mh}")
                     for mh in range(2)]

            # pass-0 feed: (z_j, A-mh0_j) pairs on the sync queue, first
            # chunk split so the first matmul starts ~5us earlier.
            nc.sync.dma_start(out=z_sb[0][:, :2 * C], in_=z0r_d[0][:, :2 * C])
            nc.sync.dma_start(out=ath_sb[0, 0][:, :2 * 512],
                              in_=atr_d[0][:, :2 * 512])
            nc.sync.dma_start(out=z_sb[0][:, 2 * C:], in_=z0r_d[0][:, 2 * C:])
            nc.sync.dma_start(out=ath_sb[0, 0][:, 2 * 512:],
                              in_=atr_d[0][:, 2 * 512:])
            for j in range(1, KCH):
                nc.sync.dma_start(out=z_sb[j][:], in_=z0r_d[j])
                nc.sync.dma_start(out=ath_sb[0, j][:], in_=atr_d[j])

            def z0_tile(t, ch):
                """lhsT: Z0[n-tile t, c-half ch] -> [128, 128] bf16."""
                j, tt = divmod(t, TPC)
                return z_sb[j][:, tt * C + ch * 128: tt * C + ch * 128 + 128]

            def z1_tile(t, ch):
                """lhsT: gathered Z1[n-tile t, c-half ch] -> [128, 128]."""
                cb, r = divmod(t, MT)
                g, i = divmod(r, MPG)
                base = cb * (MPG * C) + i * C + ch * 128
                return z1_sb[g][:, base: base + 128]

            def at_tile(t, mh):
                """rhs: A^T[n-tile t, m-half mh] -> [128, 512] bf16."""
                j, tt = divmod(t, TPC)
                return ath_sb[mh, j][:, tt * 512:(tt + 1) * 512]

            def half_tail(li, mh, h_ps, snd, hook, last):
                """Per-m-tile pipelined copy/weight-apply/relu into the
                half's contiguous send tile, then the caller's hook."""
                prio = contextlib.nullcontext() if last else tc.high_priority()
                with prio:
                    h_sb_mh = [hsb_pool.tile([128, 512], dt.bfloat16,
                                             tag=f"h{ch}",
                                             name=f"h_sb_{li}_{ch}{mh}")
                               for ch in range(2)]
                    for io in range(MPG):
                        sl = slice(io * 128, (io + 1) * 128)
                        for ch in range(2):
                            nc.vector.tensor_copy(h_sb_mh[ch][:, sl],
                                                  h_ps[ch, mh][:, sl])
                        z_ps = psz_pool.tile([128, C], dt.float32, tag="zps",
                                             name=f"z_ps_{li}_{mh}{io}")
                        for ch in range(2):
                            last_w = nc.tensor.matmul(
                                z_ps[:, ch * 128:(ch + 1) * 128],
                                h_sb_mh[ch][:, sl],
                                w_sb[:, li * 128:(li + 1) * 128],
                                start=True, stop=True)
                        nc.scalar.activation(
                            snd[mh][:, io * C:(io + 1) * C], z_ps[:],
                            mybir.ActivationFunctionType.Relu)
                        # store per m-tile: earlier pieces overlap the
                        # remaining relus, shortening the post-tail
                        # latency to the gather doorbell / kernel end
                        hook(mh, io, io + 1)
                    return last_w

            # ---- layer 1: two m-half passes in DMA arrival order ----
            def l1_hook(mh, lo, hi):
                # 2KiB-line store of part of the half's z1; after the
                # last part, the gather and the per-core-block restage
                # into SBUF.  mh=1's stores go on the gpsimd queue: its
                # DMA-completion semaphore pool is untouched by the
                # load/restage traffic, so the stores cannot inherit a
                # ring-wait on a gather-gated DMA (observed on both the
                # sync and scalar queues, delaying gather-1 by ~35us).
                st_eng = nc.sync if mh == 0 else nc.gpsimd
                st_eng.dma_start(out=z1_loc[mh][:, lo * C:hi * C],
                                 in_=z1snd[mh][:, lo * C:hi * C])
                if hi < MPG:
                    return
                nc.gpsimd.collective_compute(
                    "AllGather",
                    mybir.AluOpType.bypass,
                    replica_groups=[list(range(NCORES))],
                    ins=[z1_loc[mh].ap().opt()],
                    outs=[z1g[mh].ap().opt()],
                )
                # First core-block in two halves so layer 2's first
                # matmul sees minimum staging latency.  g0's pair is
                # deferred to after gather-1's doorbell, behind a
                # scheduler fence, and rides the gpsimd queue: it fires
                # the instant gather-0 completes, with no sync-queue
                # trigger serialization and no chance of head-of-line
                # blocking the mh1 stores / doorbell-1... (fenced).
                if mh == 1:
                    tc.no_sync_barrier()
                    for g, eng in ((0, nc.gpsimd), (1, nc.sync)):
                        eng.dma_start(
                            out=z1_sb[g][:, :MPG * C // 2],
                            in_=z1g[g].ap()[0:128, :MPG * C // 2])
                        eng.dma_start(
                            out=z1_sb[g][:, MPG * C // 2:MPG * C],
                            in_=z1g[g].ap()[0:128, MPG * C // 2:])
                for cb in range(1, NCORES):
                    nc.sync.dma_start(
                        out=z1_sb[mh][:, cb * MPG * C:(cb + 1) * MPG * C],
                        in_=z1g[mh].ap()[cb * 128:(cb + 1) * 128, :])

            h_ps1 = {(ch, mh): psh_pool.tile([128, 512], dt.float32,
                                             tag=f"hps{ch}",
                                             name=f"hps_1_{ch}{mh}")
                     for ch in range(2) for mh in range(2)}
            l1_tail_last = {}
            for mh in range(2):
                for t in range(NT):
                    for ch in range(2):
                        nc.tensor.matmul(
                            h_ps1[ch, mh][:], z0_tile(t, ch), at_tile(t, mh),
                            start=(t == 0), stop=(t == NT - 1))
                if mh == 0:
                    # first two A-mh1 chunks ahead of the mh0 z1 stores
                    # (pass 1 needs them before the stores' data exists)
                    for j in range(2):
                        nc.sync.dma_start(out=ath_sb[1, j][:],
                                          in_=atr_d[KCH + j])
                l1_tail_last[mh] = half_tail(0, mh, h_ps1, z1snd, l1_hook,
                                             last=False)
                if mh == 0:
                    # rest of the A-mh1 stream AFTER the mh0 z1 stores so
                    # the stores' packets do not trail the 20MiB load
                    for j in range(2, KCH):
                        nc.sync.dma_start(out=ath_sb[1, j][:],
                                          in_=atr_d[KCH + j])

            # ---- layer 2: gather arrival order, staggered final sweep ----
            def l2_hook(mh, lo, hi):
                base = mh * MPG * C
                nc.sync.dma_start(
                    out=out_d[:, base + lo * C:base + hi * C],
                    in_=z2snd[mh][:, lo * C:hi * C])

            t2 = [cb * MT + g * MPG + i
                  for g in range(NG) for cb in range(NCORES) for i in range(MPG)]
            h_ps2 = {(ch, mh): psh_pool.tile([128, 512], dt.float32,
                                             tag=f"hps{ch}",
                                             name=f"hps_2_{ch}{mh}")
                     for ch in range(2) for mh in range(2)}
            head, sweep = t2[:NT - STAG2], t2[NT - STAG2:]
            # Scheduler fence: layer 2's first matmul stalls on the g0
            # restage, and without this the scheduler places it on the
            # in-order tensor queue AHEAD of layer-1 mh1's weight-apply,
            # head-of-line blocking the second gather's doorbell chain
            # for ~35us (observed).  The fence orders every engine's
            # queue at this point without adding runtime semaphores.
            tc.no_sync_barrier()
            for ti, t in enumerate(head):
                for ch in range(2):
                    for mh in range(2):
                        nc.tensor.matmul(
                            h_ps2[ch, mh][:], z1_tile(t, ch), at_tile(t, mh),
                            start=(ti == 0), stop=False)
            for mh in range(2):
                for si, t in enumerate(sweep):
                    for ch in range(2):
                        nc.tensor.matmul(
                            h_ps2[ch, mh][:], z1_tile(t, ch), at_tile(t, mh),
                            start=False, stop=(si == STAG2 - 1))
                half_tail(1, mh, h_ps2, z2snd, l2_hook, last=(mh == 1))

    nc.compile()
    return nc


def _prep_inputs(x, net_params, A):
    a_bf = A.astype(BF16)
    z0 = np.ascontiguousarray(x.transpose(1, 0, 2).reshape(N, C)).astype(BF16)
    # z0 repack: [chunk j][partition p][tile tt][c]
    z0r = np.ascontiguousarray(
        z0.reshape(KCH, TPC, 128, C).transpose(0, 2, 1, 3)
    ).reshape(KCH, 128, TPC * C)
    w = net_params.astype(np.float32).reshape(L, F, F).astype(BF16)
    # block-diagonal weight tile per layer: diag(W_l, W_l)
    w_sb = np.zeros((128, 2 * 128), dtype=BF16)
    for li in range(L):
        w_sb[0:F, li * 128:li * 128 + F] = w[li]
        w_sb[F:2 * F, li * 128 + F:li * 128 + 2 * F] = w[li]
    in_maps = []
    for j in range(NCORES):
        at_j = a_bf[j * M_CORE:(j + 1) * M_CORE, :].T   # [N, M_CORE]
        # repack: [mh][chunk j][partition p][tile tt][m]
        atr = np.ascontiguousarray(
            at_j.reshape(KCH, TPC, 128, 2, 512).transpose(3, 0, 2, 1, 4)
        ).reshape(2 * KCH, 128, TPC * 512)
        in_maps.append({"atr": atr, "z0r": z0r, "w": w_sb})
    return in_maps


def kernel(x, t, net_params, A):
    x = np.asarray(x)
    A = np.asarray(A)
    net_params = np.asarray(net_params)

    if "nc" not in _CACHED:
        _CACHED["nc"] = _build_program()
    nc = _CACHED["nc"]

    in_maps = _prep_inputs(x, net_params, A)
    _CACHED["in_maps"] = in_maps
    res = run_bass_kernel_spmd(nc, in_maps, list(range(NCORES)))
    # out per core: [128, MT*C] in (p, t, c) order -> [M_CORE, C]
    parts = []
    for c in range(NCORES):
        o = np.asarray(res.results[c]["out"]).reshape(128, MT, C)
        parts.append(np.ascontiguousarray(o.transpose(1, 0, 2)).reshape(M_CORE, C))
    full = np.concatenate(parts, axis=0).astype(np.float32)
    return np.ascontiguousarray(full.reshape(N, B, F).transpose(1, 0, 2))
